# revision 16
# baseline (speedup 1.0000x reference)
"""Trainium2 Bass kernel for CausalSequenceCML — zA-factored FIR + 2x DVE.

Math (per step, grid laid out (B, C, T), sq[t] = (g[t]-0.5)^2):
    g'[t] = D[t] - C3*sq[t] - C2*sq[t-1] - C1*sq[t-2] - C0*sq[t-3]
    D     = beta*x0 + 0.25*(C0+C1+C2+C3)        (constant across steps)
    left boundary: sq[t<0] == 0.25 (zero-padded `mapped` in the reference)

Key factorization (per channel, host-solved cubic):
    P[t] = sum_j C_j sq[t-j] = alpha*zA[t] + beta_*zA[t-1]
    zA[t] = sq[t] + mu*sq[t-1] + nu*sq[t-2]
with alpha = C3, and (mu, nu, beta_) a real root of the matching system.
This splits the 4-tap FIR into:
  - opA: zA from sq — SINGLE-stream => runs in DVE 2x_2p perf mode at
    2 elem/cycle (custom uop program, both ports on one AP).
  - opB: sq'[t] = ((D-0.5)[t] - (boa*(alpha*zA[t-1]) + alpha*zA[t]))^2 —
    2-stream 1x op with the SQUARE FUSED (ScalarE's per-step square over
    the DVE region disappears). Final step: variant without square, +0.5.

2x_2p mechanics (probed on HW): the engine splits the (flattened) out/in
stream in half; port0/SRC_0 computes the first half (-> write0_lo),
port1/SRC_1 the second (-> write1_lo). Programs duplicate the block chain
per half. 1-element delays come from chain<-CURR_ALU_OUT injection
(measured age-1 in element units at both 1x and 2x); they carry across
the half boundary seamlessly EXCEPT the first 2 outputs of port1. Fix:
the out/in APs are folded as [128, 2, W] rows overlapping by 2 columns —
port0's tail overwrites port1's glitched head (port0 writes those
columns thousands of cycles later).

Pad maintenance: sq pads (cols t<0) must stay 0.25 every step, but opB
rewrites them; the pad update is itself chaotic (errors x~3.9/step). The
D'-buffer pad columns are set on host to fl(P_pad) - 0.5 replicating the
device's exact fp32 rounding sequence, making 0.25 an EXACT fixed point.
Warmup/boundary junk erodes the clean pad region by 3 cols/step from the
left; PADC=64 lead cols absorb 16 steps of erosion.

PE (TensorE) handles columns [x_s, T): 4 diag-weight tap matmuls + 2
fp32r identity matmuls adding D = Dhi+Dlo (baseline PE_R_D scheme).
ScalarE squares the PE region directly from PSUM (bias -0.5). The last
R_STEPS steps run PE taps in fp32r (weights+moving bitcast; 4+2 -> 1+1
cyc/col per tap pair) with a larger PE share X_LATE; rounding error
2^-12/step amplified ~1.9^k stays within budget for R_STEPS<=6
(simulated: rel max 4.2e-3 at R=4, 7.9e-3 at R=6; gate 2e-2).
"""

import copy

import numpy as np

from concourse import bacc, mybir
import concourse.tile as tile
import concourse.dve_ops as _dve_ops_mod
from concourse.ap import AP
from concourse.bass_utils import run_bass_kernel_spmd
from concourse.dve_spec import Spec, Src0, Src1, C0 as _SC0, C1 as _SC1
from concourse.dve_uop import (
    AluInp, AluOp, DelayInp, DveOpSpec, InpSel, OutPath, OutSel, Trigger,
    UopConfig,
)

B, T, C = 4, 4096, 512
N_CORES = 8
CPC = C // N_CORES          # channels per core = 64
ROWS = B * CPC              # 256 rows per core
HALVES = ROWS // 128        # 2 SBUF tiles per core
CLAMP = 1e-4
F32 = mybir.dt.float32
F32R = mybir.dt.float32r

PADC = 64                   # lead pad cols (erosion: ~3/step + warmup)
STEPS = 16
X_EARLY = 3264              # DVE/PE split, fp32-PE steps
X_LATE = 2304               # DVE/PE split, fp32r-PE steps
R_STEPS = 6                 # trailing steps with fp32r PE taps
PE_BLOCK = 512
MAXX = 3392                 # allocation bound for zA/D' widths (x_early <= MAXX)
XLMIN = 2304                # allocation bound for fp32r-side tiles (x_late >= XLMIN)

_compiled = {}


# --- custom DVE uop programs ------------------------------------------------

def _mkuop(rd1: bool) -> UopConfig:
    u = UopConfig()
    u.trigger = (Trigger.SRC_TENSOR_DONE, Trigger.NONE, Trigger.NONE)
    u.require_inp0 = 1
    u.require_inp1 = 1 if rd1 else 0
    return u


def _marker1x(mul: bool) -> UopConfig:
    # regular/2x_1p slots (never selected for these call sites; fp32-only)
    u = _mkuop(False)
    u.enable_input(InpSel.SRC_0, 0)
    u.datapath_config[0].enable_alu(
        AluOp.MULTIPLY if mul else AluOp.ADD,
        AluInp.PREV_ALU_OUT, AluInp.PREV_ALU_OUT)
    for k in range(1, 8):
        u.datapath_config[k].pass_through_alu()
    u.enable_output(OutSel.ALU_OUT, OutPath.WR0_LO)
    return u


def _opa_2x2p() -> UopConfig:
    """zA[t] = ((S[t] + m1[t-1]) + m2[t-1]); m1 = s0*S, m2 = s1*m1[t-1].
    s0 = mu, s1 = nu/mu. Dual per-half chains (SRC_0 -> write0 via chain0,
    SRC_1 -> write1 via ALU spine)."""
    u = _mkuop(True)      # cf requires_src1: port1 active in 2p mode
    u.enable_input(InpSel.SRC_0, 1)    # ch0
    u.enable_input(InpSel.SRC_1, 2)    # ch1
    u.enable_input(InpSel.CONST_0, 3)  # ch2 = mu
    u.enable_input(InpSel.CONST_1, 4)  # ch3 = nu/mu
    dp = u.datapath_config
    dp[0].enable_alu(AluOp.MULTIPLY, AluInp.PREV_DELAY_2, AluInp.PREV_DELAY_0)
    dp[0].enable_delay_from_src(DelayInp.CURR_ALU_OUT, 4)   # m1A age1
    dp[0].pass_through_delay(0, 1, 2, 3)
    dp[1].enable_alu(AluOp.MULTIPLY, AluInp.PREV_DELAY_3, AluInp.PREV_DELAY_4)
    dp[1].enable_delay_from_src(DelayInp.CURR_ALU_OUT, 5)   # m2A age1
    dp[1].pass_through_delay(0, 1, 2, 3, 4)
    dp[2].enable_alu(AluOp.ADD, AluInp.PREV_DELAY_0, AluInp.PREV_DELAY_4)
    dp[2].pass_through_delay(1, 2, 3, 5)
    dp[3].enable_alu(AluOp.ADD, AluInp.PREV_ALU_OUT, AluInp.PREV_DELAY_5)
    dp[3].pass_through_delay(1, 2, 3)
    dp[4].enable_alu(AluOp.MULTIPLY, AluInp.PREV_DELAY_2, AluInp.PREV_DELAY_1)
    dp[4].enable_delay_from_src(DelayInp.CURR_ALU_OUT, 4)   # m1B age1
    dp[4].enable_delay_from_src(DelayInp.PREV_ALU_OUT, 0)   # zA-A carry
    dp[4].pass_through_delay(1, 3)
    dp[5].enable_alu(AluOp.MULTIPLY, AluInp.PREV_DELAY_3, AluInp.PREV_DELAY_4)
    dp[5].enable_delay_from_src(DelayInp.CURR_ALU_OUT, 5)   # m2B age1
    dp[5].pass_through_delay(0, 1, 4)
    dp[6].enable_alu(AluOp.ADD, AluInp.PREV_DELAY_1, AluInp.PREV_DELAY_4)
    dp[6].pass_through_delay(0, 5)
    dp[7].enable_alu(AluOp.ADD, AluInp.PREV_ALU_OUT, AluInp.PREV_DELAY_5)
    dp[7].pass_through_delay(0)
    u.enable_output(OutSel.DELAY_0, OutPath.WR0_LO)
    u.enable_output(OutSel.ALU_OUT, OutPath.WR1_LO)
    return u


def _opb_1x(square: bool) -> UopConfig:
    """m = s0*in0[t]; w = s1*m[t-1]; p = w + m; d = in1[t] - p;
    out = d*d (square) or d + imm2 (final)."""
    u = _mkuop(True)
    u.enable_input(InpSel.SRC_0, 1)    # ch0 = zA
    u.enable_input(InpSel.SRC_1, 2)    # ch1 = D'
    u.enable_input(InpSel.CONST_0, 4)  # ch3 = alpha
    u.enable_input(InpSel.CONST_1, 5)  # ch4 = beta_/alpha
    if not square:
        u.enable_input(InpSel.CONST_2, 6)  # ch5 = imm2 (+0.5)
    dp = u.datapath_config
    dp[0].enable_alu(AluOp.MULTIPLY, AluInp.PREV_DELAY_0, AluInp.PREV_DELAY_3)
    dp[0].enable_delay_from_src(DelayInp.CURR_ALU_OUT, 0)   # m age1
    dp[0].pass_through_delay(1, 4, *(() if square else (5,)))
    dp[1].enable_alu(AluOp.MULTIPLY, AluInp.PREV_DELAY_0, AluInp.PREV_DELAY_4)
    dp[1].enable_delay_from_src(DelayInp.PREV_ALU_OUT, 0)   # m age0
    dp[1].pass_through_delay(1, *(() if square else (5,)))
    dp[2].enable_alu(AluOp.ADD, AluInp.PREV_ALU_OUT, AluInp.PREV_DELAY_0)
    dp[2].pass_through_delay(1, *(() if square else (5,)))
    dp[3].enable_alu(AluOp.SUBTRACT, AluInp.PREV_DELAY_1, AluInp.PREV_ALU_OUT)
    if not square:
        dp[3].pass_through_delay(5)
    if square:
        dp[4].enable_alu(AluOp.MULTIPLY, AluInp.PREV_ALU_OUT,
                         AluInp.PREV_ALU_OUT)
    else:
        dp[4].enable_alu(AluOp.ADD, AluInp.PREV_ALU_OUT, AluInp.PREV_DELAY_5)
    for k in range(5, 8):
        dp[k].pass_through_alu()
    u.enable_output(OutSel.ALU_OUT, OutPath.WR0_LO)
    return u


def _clmp_2x() -> UopConfig:
    """out = min(max(S, s0), s1), dual per-half chains (no delays)."""
    u = _mkuop(True)
    u.enable_input(InpSel.SRC_0, 1)    # ch0
    u.enable_input(InpSel.SRC_1, 2)    # ch1
    u.enable_input(InpSel.CONST_0, 3)  # ch2 = lo
    u.enable_input(InpSel.CONST_1, 4)  # ch3 = hi
    dp = u.datapath_config
    dp[0].enable_alu(AluOp.MAX, AluInp.PREV_DELAY_0, AluInp.PREV_DELAY_2)
    dp[0].pass_through_delay(1, 2, 3)
    dp[1].enable_alu(AluOp.MIN, AluInp.PREV_ALU_OUT, AluInp.PREV_DELAY_3)
    dp[1].pass_through_delay(1, 2, 3)
    dp[2].enable_alu(AluOp.MAX, AluInp.PREV_DELAY_1, AluInp.PREV_DELAY_2)
    dp[2].enable_delay_from_src(DelayInp.PREV_ALU_OUT, 4)   # outA carry
    dp[2].pass_through_delay(3)
    dp[3].enable_alu(AluOp.MIN, AluInp.PREV_ALU_OUT, AluInp.PREV_DELAY_3)
    dp[3].pass_through_delay(4)
    for k in range(4, 8):
        dp[k].pass_through_alu()
        dp[k].pass_through_delay(4)
    u.enable_output(OutSel.DELAY_4, OutPath.WR0_LO)
    u.enable_output(OutSel.ALU_OUT, OutPath.WR1_LO)
    return u


class _AntOp:
    """Duck-types concourse.dve_ops.DveOp."""

    def __init__(self, name, uop, rd1, perf, spec):
        self.name = name
        self.spec = spec
        self.rd1 = rd1
        self.subdim = False
        self._uop = uop
        self._perf = perf

    def compile(self, ver):
        assert ver == "v3"
        if self._perf:
            return DveOpSpec(
                name=self.name,
                opcode=_dve_ops_mod.get_dve_sub_opcode(self.name),
                uops=[_marker1x(False)], uops_2x=[_marker1x(True)],
                uops_2x_2p=[self._uop], uops_4x=None,
                perf_max=2, rd1_en=self.rd1)
        return DveOpSpec(
            name=self.name, opcode=_dve_ops_mod.get_dve_sub_opcode(self.name),
            uops=[self._uop], rd1_en=self.rd1)


def _register(op):
    existing = next((o for o in _dve_ops_mod.OPS if o.name == op.name), None)
    if existing is not None:
        return existing
    _dve_ops_mod.OPS.append(op)
    row = _dve_ops_mod._CUSTOM_DVE_ROW_BASE + len(_dve_ops_mod.OPS) - 1
    assert row < 0x20, row
    _dve_ops_mod._SUB_OPCODE_FOR_NAME[op.name] = row
    _dve_ops_mod.CUSTOM_DVE_SPECS[op.name] = op.spec
    return op


_SPECA = Spec(body=Src0 + Src0 * _SC0 + Src0 * _SC1,
              reference=lambda in0, in1, s0, s1, imm2:
                  in0 + in0 * s0 + in0 * s1)
_SPECB = Spec(body=Src1 - (Src0 + Src0 * _SC1) * _SC0,
              reference=lambda in0, in1, s0, s1, imm2:
                  in1 - (in0 + in0 * s1) * s0)

_ops = {}


def _get_ops():
    if not _ops:
        _ops["A"] = _register(_AntOp("ZA2X_ANT", _opa_2x2p(), False, True,
                                     _SPECA))
        _ops["B"] = _register(_AntOp("SQB1X_ANT", _opb_1x(True), True, False,
                                     _SPECB))
        _ops["F"] = _register(_AntOp("GFB1X_ANT", _opb_1x(False), True, False,
                                     _SPECB))
        _ops["C"] = _register(_AntOp("CLMP2X_ANT", _clmp_2x(), False, True,
                                     _SPECA))
    return _ops


def _fold2(t, col0, w):
    """[128, 2, w] AP over tile t: row0 at col0, row1 at col0 + w - 2
    (2-col overlap; port0's tail overwrites port1's glitched head)."""
    full = t[:, :]
    pstride = full.ap[0][0]
    return AP(full.tensor, full.offset + col0,
              [[pstride, 128], [w - 2, 2], [1, w]])


# --- kernel build -----------------------------------------------------------

def _build(steps: int, loop_k: int | None = None, x_early: int | None = None,
           x_late: int | None = None, r_steps: int | None = None):
    x_early = X_EARLY if x_early is None else x_early
    x_late = X_LATE if x_late is None else x_late
    r_steps = R_STEPS if r_steps is None else r_steps
    ops = _get_ops()
    assert x_early <= MAXX and (r_steps == 0 or x_late >= XLMIN)
    x_min = min(x_early, x_late if r_steps > 0 else x_early)
    pe_w = T - XLMIN            # Dhi/Dlo span (allocation bound)
    sqr_w = T - XLMIN + 4       # fp32r sq staging span; origin x_late-4
    nc = bacc.Bacc("TRN2", target_bir_lowering=False, debug=False)

    x_in = nc.dram_tensor("x", [ROWS, T], F32, kind="ExternalInput").ap()
    coef = nc.dram_tensor("coef", [ROWS, 10], F32, kind="ExternalInput").ap()
    wdiag = nc.dram_tensor("wdiag", [ROWS, 640], F32,
                           kind="ExternalInput").ap()
    out = nc.dram_tensor("out", [ROWS, T], F32, kind="ExternalOutput").ap()

    x_h = x_in.rearrange("(h p) t -> h p t", p=128)
    out_h = out.rearrange("(h p) t -> h p t", p=128)
    coef_h = coef.rearrange("(h p) c -> h p c", p=128)
    wdiag_h = wdiag.rearrange("(h p) c -> h p c", p=128)

    mult = mybir.AluOpType.mult
    add = mybir.AluOpType.add

    with tile.TileContext(nc) as tc:
        with tc.tile_pool(name="state", bufs=1) as pool, \
             tc.tile_pool(name="psum", bufs=8, space="PSUM") as pspool:
            neg_half = pool.tile([128, 1], F32, tag="nh", name="nh")
            nc.vector.memset(neg_half[:], -0.5)
            sqq, zab, dpb, cf, wd = [], [], [], [], []
            wir, wdr, Dhi, Dlo, sqr = [], [], [], [], []
            for h in range(HALVES):
                sqq.append([
                    pool.tile([128, PADC + T], F32, tag=f"sqa{h}",
                              name=f"sqa{h}"),
                    pool.tile([128, PADC + T], F32, tag=f"sqb{h}",
                              name=f"sqb{h}"),
                ])
                zab.append(pool.tile([128, PADC + MAXX], F32, tag=f"za{h}",
                                     name=f"za{h}"))
                dpb.append(pool.tile([128, PADC + MAXX], F32, tag=f"dp{h}",
                                     name=f"dp{h}"))
                cf.append(pool.tile([128, 10], F32, tag=f"cf{h}", name=f"cf{h}"))
                wd.append(pool.tile([128, 640], F32, tag=f"wd{h}",
                                    name=f"wd{h}"))
                wir.append(pool.tile([128, 128], F32R, tag=f"wir{h}",
                                     name=f"wir{h}"))
                Dhi.append(pool.tile([128, pe_w], F32R, tag=f"dh{h}",
                                     name=f"dh{h}"))
                Dlo.append(pool.tile([128, pe_w], F32R, tag=f"dl{h}",
                                     name=f"dl{h}"))
                if r_steps > 0:
                    wdr.append(pool.tile([128, 512], F32R, tag=f"wdr{h}",
                                         name=f"wdr{h}"))
                    sqr.append([
                        pool.tile([128, sqr_w], F32R, tag=f"sra{h}",
                                  name=f"sra{h}"),
                        pool.tile([128, sqr_w], F32R, tag=f"srb{h}",
                                  name=f"srb{h}"),
                    ])

            for h in range(HALVES):
                nc.sync.dma_start(out=cf[h][:], in_=coef_h[h])
                nc.sync.dma_start(out=wd[h][:], in_=wdiag_h[h])
                # x lands in sqB's data region (rewritten at step 0)
                nc.sync.dma_start(out=sqq[h][1][:, PADC:PADC + T], in_=x_h[h])
                nc.scalar.copy(wir[h][:], wd[h][:, 512:640])
                if r_steps > 0:
                    nc.scalar.copy(wdr[h][:], wd[h][:, 0:512])
                for p in range(2):
                    nc.vector.memset(sqq[h][p][:, 0:PADC], 0.25)
                nc.vector.memset(zab[h][:, 0:PADC], 0.0)
                xsrc = sqq[h][1][:, PADC:PADC + T]
                # D' = beta*x0 + (dconst - 0.5); pads = host dpad column
                nc.vector.tensor_scalar(
                    dpb[h][:, PADC:PADC + MAXX], xsrc[:, 0:MAXX],
                    cf[h][:, 4:5], cf[h][:, 5:6], mult, add)
                nc.vector.tensor_scalar(
                    dpb[h][:, 0:PADC], sqq[h][0][:, 0:PADC],
                    0.0, cf[h][:, 6:7], mult, add)
                # D true over the PE span -> Dhi/Dlo (fp32r split); sqA data
                # region used as init scratch (sq_0 overwrites it after)
                scr = sqq[h][0][:, PADC:PADC + pe_w]
                nc.vector.tensor_scalar(
                    scr, xsrc[:, XLMIN:T],
                    cf[h][:, 4:5], cf[h][:, 7:8], mult, add)
                nc.scalar.copy(Dhi[h][:], scr)
                nc.vector.tensor_tensor(
                    Dlo[h][:], scr, Dhi[h][:].bitcast(F32),
                    mybir.AluOpType.subtract)
                # sq_0 = (x - 0.5)^2
                nc.scalar.activation(
                    sqq[h][0][:, PADC:PADC + T], xsrc,
                    mybir.ActivationFunctionType.Square, bias=neg_half[:])

            def emit_step(s, final):
                use_r = r_steps > 0 and s >= steps - r_steps
                nxt_r = (r_steps > 0 and not final
                         and (s + 1) >= steps - r_steps)
                xs = x_late if use_r else x_early
                sq0 = x_late - 4          # sqr tile origin (main-sq col)
                for h in range(HALVES):
                    cur = sqq[h][s % 2]
                    nxt = sqq[h][(s + 1) % 2]
                    # opA: zA over cols [2, PADC+xs), folded 2x
                    L = PADC + xs - 2
                    W = (L + 2) // 2
                    bi = nc.vector._custom_dve(
                        ops["A"], out=_fold2(zab[h], 2, W),
                        in0=_fold2(cur, 2, W),
                        s0=cf[h][:, 0:1], s1=cf[h][:, 1:2])
                    bi.ins.perf_max = 2
                    # PE region: ps = sum -Cj sq[t-j] + D
                    blocks = []
                    c = xs
                    while c < T:
                        n = min(PE_BLOCK, T - c)
                        ps = pspool.tile([128, PE_BLOCK], F32, tag="ps",
                                         name=f"ps{s}_{h}_{c}")
                        for k in range(4):
                            if use_r:
                                # moving operand from the fp32r staging tile
                                nc.tensor.matmul(
                                    ps[:, :n],
                                    wdr[h][:, k * 128:(k + 1) * 128],
                                    sqr[h][s % 2][:, c - k - sq0:
                                                  c - k - sq0 + n],
                                    start=(k == 0), stop=False)
                            else:
                                nc.tensor.matmul(
                                    ps[:, :n],
                                    wd[h][:, k * 128:(k + 1) * 128],
                                    cur[:, PADC - k + c:PADC - k + c + n],
                                    start=(k == 0), stop=False)
                        rc = c - XLMIN
                        nc.tensor.matmul(ps[:, :n], wir[h][:],
                                         Dhi[h][:, rc:rc + n],
                                         start=False, stop=False)
                        nc.tensor.matmul(ps[:, :n], wir[h][:],
                                         Dlo[h][:, rc:rc + n],
                                         start=False, stop=True)
                        blocks.append((c, n, ps))
                        c += n
                    # opB: sq' (or final g') over cols [3, PADC+xs)
                    nc.vector._custom_dve(
                        ops["F" if final else "B"],
                        out=nxt[:, 3:PADC + xs],
                        in0=zab[h][:, 3:PADC + xs],
                        in1=dpb[h][:, 3:PADC + xs],
                        s0=cf[h][:, 2:3], s1=cf[h][:, 3:4],
                        imm2=0.5 if final else 0.0)
                    # ScalarE: PE region square from PSUM (or copy on final).
                    # When the NEXT step uses fp32r taps, the square goes to
                    # the fp32r staging tile instead of the main sq buffer,
                    # plus a round-copy of the opB-written strip
                    # [x_next-3, xs) so taps below x_next are covered.
                    for (c, n, ps) in blocks:
                        if final:
                            nc.scalar.copy(nxt[:, PADC + c:PADC + c + n],
                                           ps[:, :n])
                        elif nxt_r:
                            nc.scalar.activation(
                                sqr[h][(s + 1) % 2][:, c - sq0:c - sq0 + n],
                                ps[:, :n],
                                mybir.ActivationFunctionType.Square,
                                bias=neg_half[:])
                        else:
                            nc.scalar.activation(
                                nxt[:, PADC + c:PADC + c + n], ps[:, :n],
                                mybir.ActivationFunctionType.Square,
                                bias=neg_half[:])
                    if nxt_r:
                        # strip [x_late-3, xs): rounded copy of opB's output
                        nc.scalar.copy(
                            sqr[h][(s + 1) % 2][:, 1:1 + (xs - x_late + 3)],
                            nxt[:, PADC + x_late - 3:PADC + xs])

            if loop_k is not None:
                with tc.For_i(0, loop_k):
                    for s in range(steps):
                        emit_step(s, final=False)
            else:
                for s in range(steps):
                    emit_step(s, final=(s == steps - 1))

            fin = [sqq[h][steps % 2] for h in range(HALVES)]
            for h in range(HALVES):
                nc.vector.tensor_scalar(
                    fin[h][:, PADC:PADC + T], fin[h][:, PADC:PADC + T],
                    CLAMP, 1.0 - CLAMP,
                    mybir.AluOpType.max, mybir.AluOpType.min)
                nc.sync.dma_start(out=out_h[h], in_=fin[h][:, PADC:PADC + T])

    nc.compile()
    return nc


def get_nc(steps: int):
    if steps not in _compiled:
        _compiled[steps] = _build(steps)
    return _compiled[steps]


# --- host prep --------------------------------------------------------------

def _solve_za(C0, C1, C2, C3):
    """Per-channel real root of the zA-factorization cubic."""
    n = len(C0)
    mus = np.zeros(n); nus = np.zeros(n); bets = np.zeros(n)
    for c in range(n):
        a3, a2, a1, a0 = C3[c], C2[c], C1[c], C0[c]
        coeffs = [a3 ** 2, -2 * a2 * a3, a2 ** 2 + a1 * a3, a3 * a0 - a1 * a2]
        best = None
        for rt in np.roots(coeffs):
            if abs(rt.imag) > 1e-9 * max(1.0, abs(rt.real)):
                continue
            mu = rt.real
            b_ = a2 - a3 * mu
            if abs(b_) < 1e-9 or abs(mu) < 1e-7:
                continue
            nu = a0 / b_
            cond = abs(mu) + abs(nu) + abs(b_ / a3) + abs(nu / mu)
            if best is None or cond < best[0]:
                best = (cond, mu, nu, b_)
        assert best is not None, f"no usable root for channel {c}"
        _, mus[c], nus[c], bets[c] = best
    return mus, nus, bets


def _host_prep(drive, r, eps, beta, K_causal):
    """Per-core inputs: x (256,T), coef (256,8), wdiag (256,640)."""
    f32 = np.float32
    drive = np.asarray(drive, f32)
    r = np.asarray(r, np.float64)
    eps = np.asarray(eps, np.float64)
    beta = np.asarray(beta, np.float64)
    K = np.asarray(K_causal, np.float64)[:, 0, :]  # (C, 4)

    one_m_b = 1.0 - beta
    C0 = one_m_b * eps * r * K[:, 0]
    C1 = one_m_b * eps * r * K[:, 1]
    C2 = one_m_b * eps * r * K[:, 2]
    C3 = one_m_b * r * ((1.0 - eps) + eps * K[:, 3])
    dconst = 0.25 * (C0 + C1 + C2 + C3)
    mus, nus, bets = _solve_za(C0, C1, C2, C3)

    mu32 = mus.astype(f32)
    nom32 = (nus.astype(f32) / mus.astype(f32)).astype(f32)
    al32 = C3.astype(f32)
    boa32 = (bets.astype(f32) / C3.astype(f32)).astype(f32)
    # device-exact pad fixed point: D'pad = fl(P_pad) - 0.5
    q = f32(0.25)
    m1p = (mu32 * q).astype(f32)
    m2p = (nom32 * m1p).astype(f32)
    Zp = ((q + m1p).astype(f32) + m2p).astype(f32)
    mp = (al32 * Zp).astype(f32)
    wp = (boa32 * mp).astype(f32)
    Pp = (wp + mp).astype(f32)
    dpad = (Pp - f32(0.5)).astype(f32)

    in_maps = []
    idx = np.arange(128)
    for i in range(N_CORES):
        sl = slice(i * CPC, (i + 1) * CPC)
        xs = np.ascontiguousarray(
            drive[:, :, sl].transpose(0, 2, 1).reshape(ROWS, T), f32)
        cs = np.stack(
            [np.tile(mu32[sl], B), np.tile(nom32[sl], B),
             np.tile(al32[sl], B), np.tile(boa32[sl], B),
             np.tile(beta[sl].astype(f32), B),
             np.tile((dconst.astype(f32) - f32(0.5))[sl], B),
             np.tile(dpad[sl], B), np.tile(dconst.astype(f32)[sl], B),
             np.full(ROWS, CLAMP, f32), np.full(ROWS, 1.0 - CLAMP, f32)],
            axis=1).astype(f32)
        blocks = [-C3, -C2, -C1, -C0, np.ones(C)]
        wdg = np.zeros((ROWS, 640), f32)
        for k, arr in enumerate(blocks):
            rows = np.tile(arr.astype(f32)[sl], B)
            for h in range(HALVES):
                wdg[h * 128 + idx, k * 128 + idx] = rows[h * 128 + idx]
        in_maps.append({"x": xs, "coef": np.ascontiguousarray(cs),
                        "wdiag": wdg})
    return in_maps


def kernel(drive, r, eps, beta, K_causal, steps):
    steps = int(steps)
    nc = get_nc(steps)
    in_maps = _host_prep(drive, r, eps, beta, K_causal)
    res = run_bass_kernel_spmd(nc, in_maps, list(range(N_CORES)))
    parts = [
        res.results[i]["out"].reshape(B, CPC, T).transpose(0, 2, 1)
        for i in range(N_CORES)
    ]
    return np.ascontiguousarray(np.concatenate(parts, axis=2), np.float32)


# revision 17
# speedup vs baseline: 1.0505x; 1.0505x over previous
"""Trainium2 Bass kernel for CausalSequenceCML — zA-factored FIR + 2x DVE.

Math (per step, grid laid out (B, C, T), sq[t] = (g[t]-0.5)^2):
    g'[t] = D[t] - C3*sq[t] - C2*sq[t-1] - C1*sq[t-2] - C0*sq[t-3]
    D     = beta*x0 + 0.25*(C0+C1+C2+C3)        (constant across steps)
    left boundary: sq[t<0] == 0.25 (zero-padded `mapped` in the reference)

Key factorization (per channel, host-solved cubic):
    P[t] = sum_j C_j sq[t-j] = alpha*zA[t] + beta_*zA[t-1]
    zA[t] = sq[t] + mu*sq[t-1] + nu*sq[t-2]
with alpha = C3, and (mu, nu, beta_) a real root of the matching system.
This splits the 4-tap FIR into:
  - opA: zA from sq — SINGLE-stream => runs in DVE 2x_2p perf mode at
    2 elem/cycle (custom uop program, both ports on one AP).
  - opB: sq'[t] = ((D-0.5)[t] - (boa*(alpha*zA[t-1]) + alpha*zA[t]))^2 —
    2-stream 1x op with the SQUARE FUSED (ScalarE's per-step square over
    the DVE region disappears). Final step: variant without square, +0.5.

2x_2p mechanics (probed on HW): the engine splits the (flattened) out/in
stream in half; port0/SRC_0 computes the first half (-> write0_lo),
port1/SRC_1 the second (-> write1_lo). Programs duplicate the block chain
per half. 1-element delays come from chain<-CURR_ALU_OUT injection
(measured age-1 in element units at both 1x and 2x); they carry across
the half boundary seamlessly EXCEPT the first 2 outputs of port1. Fix:
the out/in APs are folded as [128, 2, W] rows overlapping by 2 columns —
port0's tail overwrites port1's glitched head (port0 writes those
columns thousands of cycles later).

Pad maintenance: sq pads (cols t<0) must stay 0.25 every step, but opB
rewrites them; the pad update is itself chaotic (errors x~3.9/step). The
D'-buffer pad columns are set on host to fl(P_pad) - 0.5 replicating the
device's exact fp32 rounding sequence, making 0.25 an EXACT fixed point.
Warmup/boundary junk erodes the clean pad region by 3 cols/step from the
left; PADC=64 lead cols absorb 16 steps of erosion.

PE (TensorE) handles columns [x_s, T): 4 diag-weight tap matmuls + 2
fp32r identity matmuls adding D = Dhi+Dlo (baseline PE_R_D scheme).
ScalarE squares the PE region directly from PSUM (bias -0.5). The last
R_STEPS steps run PE taps in fp32r (weights+moving bitcast; 4+2 -> 1+1
cyc/col per tap pair) with a larger PE share X_LATE; rounding error
2^-12/step amplified ~1.9^k stays within budget for R_STEPS<=6
(simulated: rel max 4.2e-3 at R=4, 7.9e-3 at R=6; gate 2e-2).
"""

import copy

import numpy as np

from concourse import bacc, mybir
import concourse.tile as tile
import concourse.dve_ops as _dve_ops_mod
from concourse.ap import AP
from concourse.bass_utils import run_bass_kernel_spmd
from concourse.dve_spec import Spec, Src0, Src1, C0 as _SC0, C1 as _SC1
from concourse.dve_uop import (
    AluInp, AluOp, DelayInp, DveOpSpec, InpSel, OutPath, OutSel, Trigger,
    UopConfig,
)

B, T, C = 4, 4096, 512
N_CORES = 8
CPC = C // N_CORES          # channels per core = 64
ROWS = B * CPC              # 256 rows per core
HALVES = ROWS // 128        # 2 SBUF tiles per core
CLAMP = 1e-4
F32 = mybir.dt.float32
F32R = mybir.dt.float32r

PADC = 60                   # lead pad cols (erosion: ~3/step + warmup; min 57)
STEPS = 16
X_EARLY = 3264              # DVE/PE split, fp32-PE steps
X_LATE = 2240               # DVE/PE split, fp32r-PE steps
R_STEPS = 6                 # trailing steps with fp32r PE taps
PE_BLOCK = 512
MAXX = 3328                 # allocation bound for zA/D' widths (x_early <= MAXX)
XLMIN = 2240                # allocation bound for fp32r-side tiles (x_late >= XLMIN)

_compiled = {}


# --- custom DVE uop programs ------------------------------------------------

def _mkuop(rd1: bool) -> UopConfig:
    u = UopConfig()
    u.trigger = (Trigger.SRC_TENSOR_DONE, Trigger.NONE, Trigger.NONE)
    u.require_inp0 = 1
    u.require_inp1 = 1 if rd1 else 0
    return u


def _marker1x(mul: bool) -> UopConfig:
    # regular/2x_1p slots (never selected for these call sites; fp32-only)
    u = _mkuop(False)
    u.enable_input(InpSel.SRC_0, 0)
    u.datapath_config[0].enable_alu(
        AluOp.MULTIPLY if mul else AluOp.ADD,
        AluInp.PREV_ALU_OUT, AluInp.PREV_ALU_OUT)
    for k in range(1, 8):
        u.datapath_config[k].pass_through_alu()
    u.enable_output(OutSel.ALU_OUT, OutPath.WR0_LO)
    return u


def _opa_2x2p() -> UopConfig:
    """zA[t] = ((S[t] + m1[t-1]) + m2[t-1]); m1 = s0*S, m2 = s1*m1[t-1].
    s0 = mu, s1 = nu/mu. Dual per-half chains (SRC_0 -> write0 via chain0,
    SRC_1 -> write1 via ALU spine)."""
    u = _mkuop(True)      # cf requires_src1: port1 active in 2p mode
    u.enable_input(InpSel.SRC_0, 1)    # ch0
    u.enable_input(InpSel.SRC_1, 2)    # ch1
    u.enable_input(InpSel.CONST_0, 3)  # ch2 = mu
    u.enable_input(InpSel.CONST_1, 4)  # ch3 = nu/mu
    dp = u.datapath_config
    dp[0].enable_alu(AluOp.MULTIPLY, AluInp.PREV_DELAY_2, AluInp.PREV_DELAY_0)
    dp[0].enable_delay_from_src(DelayInp.CURR_ALU_OUT, 4)   # m1A age1
    dp[0].pass_through_delay(0, 1, 2, 3)
    dp[1].enable_alu(AluOp.MULTIPLY, AluInp.PREV_DELAY_3, AluInp.PREV_DELAY_4)
    dp[1].enable_delay_from_src(DelayInp.CURR_ALU_OUT, 5)   # m2A age1
    dp[1].pass_through_delay(0, 1, 2, 3, 4)
    dp[2].enable_alu(AluOp.ADD, AluInp.PREV_DELAY_0, AluInp.PREV_DELAY_4)
    dp[2].pass_through_delay(1, 2, 3, 5)
    dp[3].enable_alu(AluOp.ADD, AluInp.PREV_ALU_OUT, AluInp.PREV_DELAY_5)
    dp[3].pass_through_delay(1, 2, 3)
    dp[4].enable_alu(AluOp.MULTIPLY, AluInp.PREV_DELAY_2, AluInp.PREV_DELAY_1)
    dp[4].enable_delay_from_src(DelayInp.CURR_ALU_OUT, 4)   # m1B age1
    dp[4].enable_delay_from_src(DelayInp.PREV_ALU_OUT, 0)   # zA-A carry
    dp[4].pass_through_delay(1, 3)
    dp[5].enable_alu(AluOp.MULTIPLY, AluInp.PREV_DELAY_3, AluInp.PREV_DELAY_4)
    dp[5].enable_delay_from_src(DelayInp.CURR_ALU_OUT, 5)   # m2B age1
    dp[5].pass_through_delay(0, 1, 4)
    dp[6].enable_alu(AluOp.ADD, AluInp.PREV_DELAY_1, AluInp.PREV_DELAY_4)
    dp[6].pass_through_delay(0, 5)
    dp[7].enable_alu(AluOp.ADD, AluInp.PREV_ALU_OUT, AluInp.PREV_DELAY_5)
    dp[7].pass_through_delay(0)
    u.enable_output(OutSel.DELAY_0, OutPath.WR0_LO)
    u.enable_output(OutSel.ALU_OUT, OutPath.WR1_LO)
    return u


def _opb_1x(square: bool) -> UopConfig:
    """m = s0*in0[t]; w = s1*m[t-1]; p = w + m; d = in1[t] - p;
    out = d*d (square) or d + imm2 (final)."""
    u = _mkuop(True)
    u.enable_input(InpSel.SRC_0, 1)    # ch0 = zA
    u.enable_input(InpSel.SRC_1, 2)    # ch1 = D'
    u.enable_input(InpSel.CONST_0, 4)  # ch3 = alpha
    u.enable_input(InpSel.CONST_1, 5)  # ch4 = beta_/alpha
    if not square:
        u.enable_input(InpSel.CONST_2, 6)  # ch5 = imm2 (+0.5)
    dp = u.datapath_config
    dp[0].enable_alu(AluOp.MULTIPLY, AluInp.PREV_DELAY_0, AluInp.PREV_DELAY_3)
    dp[0].enable_delay_from_src(DelayInp.CURR_ALU_OUT, 0)   # m age1
    dp[0].pass_through_delay(1, 4, *(() if square else (5,)))
    dp[1].enable_alu(AluOp.MULTIPLY, AluInp.PREV_DELAY_0, AluInp.PREV_DELAY_4)
    dp[1].enable_delay_from_src(DelayInp.PREV_ALU_OUT, 0)   # m age0
    dp[1].pass_through_delay(1, *(() if square else (5,)))
    dp[2].enable_alu(AluOp.ADD, AluInp.PREV_ALU_OUT, AluInp.PREV_DELAY_0)
    dp[2].pass_through_delay(1, *(() if square else (5,)))
    dp[3].enable_alu(AluOp.SUBTRACT, AluInp.PREV_DELAY_1, AluInp.PREV_ALU_OUT)
    if not square:
        dp[3].pass_through_delay(5)
    if square:
        dp[4].enable_alu(AluOp.MULTIPLY, AluInp.PREV_ALU_OUT,
                         AluInp.PREV_ALU_OUT)
    else:
        dp[4].enable_alu(AluOp.ADD, AluInp.PREV_ALU_OUT, AluInp.PREV_DELAY_5)
    for k in range(5, 8):
        dp[k].pass_through_alu()
    u.enable_output(OutSel.ALU_OUT, OutPath.WR0_LO)
    return u


def _clmp_2x() -> UopConfig:
    """out = min(max(S, s0), s1), dual per-half chains (no delays)."""
    u = _mkuop(True)
    u.enable_input(InpSel.SRC_0, 1)    # ch0
    u.enable_input(InpSel.SRC_1, 2)    # ch1
    u.enable_input(InpSel.CONST_0, 3)  # ch2 = lo
    u.enable_input(InpSel.CONST_1, 4)  # ch3 = hi
    dp = u.datapath_config
    dp[0].enable_alu(AluOp.MAX, AluInp.PREV_DELAY_0, AluInp.PREV_DELAY_2)
    dp[0].pass_through_delay(1, 2, 3)
    dp[1].enable_alu(AluOp.MIN, AluInp.PREV_ALU_OUT, AluInp.PREV_DELAY_3)
    dp[1].pass_through_delay(1, 2, 3)
    dp[2].enable_alu(AluOp.MAX, AluInp.PREV_DELAY_1, AluInp.PREV_DELAY_2)
    dp[2].enable_delay_from_src(DelayInp.PREV_ALU_OUT, 4)   # outA carry
    dp[2].pass_through_delay(3)
    dp[3].enable_alu(AluOp.MIN, AluInp.PREV_ALU_OUT, AluInp.PREV_DELAY_3)
    dp[3].pass_through_delay(4)
    for k in range(4, 8):
        dp[k].pass_through_alu()
        dp[k].pass_through_delay(4)
    u.enable_output(OutSel.DELAY_4, OutPath.WR0_LO)
    u.enable_output(OutSel.ALU_OUT, OutPath.WR1_LO)
    return u


class _AntOp:
    """Duck-types concourse.dve_ops.DveOp."""

    def __init__(self, name, uop, rd1, perf, spec):
        self.name = name
        self.spec = spec
        self.rd1 = rd1
        self.subdim = False
        self._uop = uop
        self._perf = perf

    def compile(self, ver):
        assert ver == "v3"
        if self._perf:
            return DveOpSpec(
                name=self.name,
                opcode=_dve_ops_mod.get_dve_sub_opcode(self.name),
                uops=[_marker1x(False)], uops_2x=[_marker1x(True)],
                uops_2x_2p=[self._uop], uops_4x=None,
                perf_max=2, rd1_en=self.rd1)
        return DveOpSpec(
            name=self.name, opcode=_dve_ops_mod.get_dve_sub_opcode(self.name),
            uops=[self._uop], rd1_en=self.rd1)


def _register(op):
    existing = next((o for o in _dve_ops_mod.OPS if o.name == op.name), None)
    if existing is not None:
        return existing
    _dve_ops_mod.OPS.append(op)
    row = _dve_ops_mod._CUSTOM_DVE_ROW_BASE + len(_dve_ops_mod.OPS) - 1
    assert row < 0x20, row
    _dve_ops_mod._SUB_OPCODE_FOR_NAME[op.name] = row
    _dve_ops_mod.CUSTOM_DVE_SPECS[op.name] = op.spec
    return op


_SPECA = Spec(body=Src0 + Src0 * _SC0 + Src0 * _SC1,
              reference=lambda in0, in1, s0, s1, imm2:
                  in0 + in0 * s0 + in0 * s1)
_SPECB = Spec(body=Src1 - (Src0 + Src0 * _SC1) * _SC0,
              reference=lambda in0, in1, s0, s1, imm2:
                  in1 - (in0 + in0 * s1) * s0)

_ops = {}


def _get_ops():
    if not _ops:
        _ops["A"] = _register(_AntOp("ZA2X_ANT", _opa_2x2p(), False, True,
                                     _SPECA))
        _ops["B"] = _register(_AntOp("SQB1X_ANT", _opb_1x(True), True, False,
                                     _SPECB))
        _ops["F"] = _register(_AntOp("GFB1X_ANT", _opb_1x(False), True, False,
                                     _SPECB))
        _ops["C"] = _register(_AntOp("CLMP2X_ANT", _clmp_2x(), False, True,
                                     _SPECA))
    return _ops


def _fold2(t, col0, w):
    """[128, 2, w] AP over tile t: row0 at col0, row1 at col0 + w - 2
    (2-col overlap; port0's tail overwrites port1's glitched head)."""
    full = t[:, :]
    pstride = full.ap[0][0]
    return AP(full.tensor, full.offset + col0,
              [[pstride, 128], [w - 2, 2], [1, w]])


# --- kernel build -----------------------------------------------------------

def _build(steps: int, loop_k: int | None = None, x_early: int | None = None,
           x_late: int | None = None, r_steps: int | None = None):
    x_early = X_EARLY if x_early is None else x_early
    x_late = X_LATE if x_late is None else x_late
    r_steps = R_STEPS if r_steps is None else r_steps
    ops = _get_ops()
    assert x_early <= MAXX and (r_steps == 0 or x_late >= XLMIN)
    x_min = min(x_early, x_late if r_steps > 0 else x_early)
    pe_w = T - XLMIN            # Dhi/Dlo span (allocation bound)
    sqr_w = T - XLMIN + 4       # fp32r sq staging span; origin x_late-4
    nc = bacc.Bacc("TRN2", target_bir_lowering=False, debug=False)

    x_in = nc.dram_tensor("x", [ROWS, T], F32, kind="ExternalInput").ap()
    coef = nc.dram_tensor("coef", [ROWS, 10], F32, kind="ExternalInput").ap()
    wdiag = nc.dram_tensor("wdiag", [ROWS, 640], F32,
                           kind="ExternalInput").ap()
    out = nc.dram_tensor("out", [ROWS, T], F32, kind="ExternalOutput").ap()

    x_h = x_in.rearrange("(h p) t -> h p t", p=128)
    out_h = out.rearrange("(h p) t -> h p t", p=128)
    coef_h = coef.rearrange("(h p) c -> h p c", p=128)
    wdiag_h = wdiag.rearrange("(h p) c -> h p c", p=128)

    mult = mybir.AluOpType.mult
    add = mybir.AluOpType.add

    with tile.TileContext(nc) as tc:
        with tc.tile_pool(name="state", bufs=1) as pool, \
             tc.tile_pool(name="psum", bufs=8, space="PSUM") as pspool:
            neg_half = pool.tile([128, 1], F32, tag="nh", name="nh")
            nc.vector.memset(neg_half[:], -0.5)
            sqq, zab, dpb, cf, wd = [], [], [], [], []
            wir, wdr, Dhi, Dlo, sqr = [], [], [], [], []
            for h in range(HALVES):
                sqq.append([
                    pool.tile([128, PADC + T], F32, tag=f"sqa{h}",
                              name=f"sqa{h}"),
                    pool.tile([128, PADC + T], F32, tag=f"sqb{h}",
                              name=f"sqb{h}"),
                ])
                zab.append(pool.tile([128, PADC + MAXX], F32, tag=f"za{h}",
                                     name=f"za{h}"))
                dpb.append(pool.tile([128, PADC + MAXX], F32, tag=f"dp{h}",
                                     name=f"dp{h}"))
                cf.append(pool.tile([128, 10], F32, tag=f"cf{h}", name=f"cf{h}"))
                wd.append(pool.tile([128, 640], F32, tag=f"wd{h}",
                                    name=f"wd{h}"))
                wir.append(pool.tile([128, 128], F32R, tag=f"wir{h}",
                                     name=f"wir{h}"))
                Dhi.append(pool.tile([128, pe_w], F32R, tag=f"dh{h}",
                                     name=f"dh{h}"))
                Dlo.append(pool.tile([128, pe_w], F32R, tag=f"dl{h}",
                                     name=f"dl{h}"))
                if r_steps > 0:
                    wdr.append(pool.tile([128, 512], F32R, tag=f"wdr{h}",
                                         name=f"wdr{h}"))
                    sqr.append([
                        pool.tile([128, sqr_w], F32R, tag=f"sra{h}",
                                  name=f"sra{h}"),
                        pool.tile([128, sqr_w], F32R, tag=f"srb{h}",
                                  name=f"srb{h}"),
                    ])

            for h in range(HALVES):
                nc.sync.dma_start(out=cf[h][:], in_=coef_h[h])
                nc.sync.dma_start(out=wd[h][:], in_=wdiag_h[h])
                # x lands in sqB's data region (rewritten at step 0)
                nc.sync.dma_start(out=sqq[h][1][:, PADC:PADC + T], in_=x_h[h])
                nc.scalar.copy(wir[h][:], wd[h][:, 512:640])
                if r_steps > 0:
                    nc.scalar.copy(wdr[h][:], wd[h][:, 0:512])
                for p in range(2):
                    nc.vector.memset(sqq[h][p][:, 0:PADC], 0.25)
                nc.vector.memset(zab[h][:, 0:PADC], 0.0)
                xsrc = sqq[h][1][:, PADC:PADC + T]
                # D' = beta*x0 + (dconst - 0.5); pads = host dpad column
                nc.vector.tensor_scalar(
                    dpb[h][:, PADC:PADC + MAXX], xsrc[:, 0:MAXX],
                    cf[h][:, 4:5], cf[h][:, 5:6], mult, add)
                nc.vector.tensor_scalar(
                    dpb[h][:, 0:PADC], sqq[h][0][:, 0:PADC],
                    0.0, cf[h][:, 6:7], mult, add)
                # D true over the PE span -> Dhi/Dlo (fp32r split); sqA data
                # region used as init scratch (sq_0 overwrites it after)
                scr = sqq[h][0][:, PADC:PADC + pe_w]
                nc.vector.tensor_scalar(
                    scr, xsrc[:, XLMIN:T],
                    cf[h][:, 4:5], cf[h][:, 7:8], mult, add)
                nc.scalar.copy(Dhi[h][:], scr)
                nc.vector.tensor_tensor(
                    Dlo[h][:], scr, Dhi[h][:].bitcast(F32),
                    mybir.AluOpType.subtract)
                # sq_0 = (x - 0.5)^2
                nc.scalar.activation(
                    sqq[h][0][:, PADC:PADC + T], xsrc,
                    mybir.ActivationFunctionType.Square, bias=neg_half[:])

            def emit_step(s, final):
                use_r = r_steps > 0 and s >= steps - r_steps
                nxt_r = (r_steps > 0 and not final
                         and (s + 1) >= steps - r_steps)
                xs = x_late if use_r else x_early
                sq0 = x_late - 4          # sqr tile origin (main-sq col)
                for h in range(HALVES):
                    cur = sqq[h][s % 2]
                    nxt = sqq[h][(s + 1) % 2]
                    # opA: zA over cols [2, PADC+xs), folded 2x
                    L = PADC + xs - 2
                    W = (L + 2) // 2
                    bi = nc.vector._custom_dve(
                        ops["A"], out=_fold2(zab[h], 2, W),
                        in0=_fold2(cur, 2, W),
                        s0=cf[h][:, 0:1], s1=cf[h][:, 1:2])
                    bi.ins.perf_max = 2
                    # PE region: ps = sum -Cj sq[t-j] + D
                    blocks = []
                    c = xs
                    while c < T:
                        n = min(PE_BLOCK, T - c)
                        ps = pspool.tile([128, PE_BLOCK], F32, tag="ps",
                                         name=f"ps{s}_{h}_{c}")
                        for k in range(4):
                            if use_r:
                                # moving operand from the fp32r staging tile
                                nc.tensor.matmul(
                                    ps[:, :n],
                                    wdr[h][:, k * 128:(k + 1) * 128],
                                    sqr[h][s % 2][:, c - k - sq0:
                                                  c - k - sq0 + n],
                                    start=(k == 0), stop=False)
                            else:
                                nc.tensor.matmul(
                                    ps[:, :n],
                                    wd[h][:, k * 128:(k + 1) * 128],
                                    cur[:, PADC - k + c:PADC - k + c + n],
                                    start=(k == 0), stop=False)
                        rc = c - XLMIN
                        nc.tensor.matmul(ps[:, :n], wir[h][:],
                                         Dhi[h][:, rc:rc + n],
                                         start=False, stop=False)
                        nc.tensor.matmul(ps[:, :n], wir[h][:],
                                         Dlo[h][:, rc:rc + n],
                                         start=False, stop=True)
                        blocks.append((c, n, ps))
                        c += n
                    # opB: sq' (or final g') over cols [3, PADC+xs)
                    nc.vector._custom_dve(
                        ops["F" if final else "B"],
                        out=nxt[:, 3:PADC + xs],
                        in0=zab[h][:, 3:PADC + xs],
                        in1=dpb[h][:, 3:PADC + xs],
                        s0=cf[h][:, 2:3], s1=cf[h][:, 3:4],
                        imm2=0.5 if final else 0.0)
                    # ScalarE: PE region square from PSUM (or copy on final).
                    # When the NEXT step uses fp32r taps, the square goes to
                    # the fp32r staging tile instead of the main sq buffer,
                    # plus a round-copy of the opB-written strip
                    # [x_next-3, xs) so taps below x_next are covered.
                    for (c, n, ps) in blocks:
                        if final:
                            nc.scalar.copy(nxt[:, PADC + c:PADC + c + n],
                                           ps[:, :n])
                        elif nxt_r:
                            nc.scalar.activation(
                                sqr[h][(s + 1) % 2][:, c - sq0:c - sq0 + n],
                                ps[:, :n],
                                mybir.ActivationFunctionType.Square,
                                bias=neg_half[:])
                        else:
                            nc.scalar.activation(
                                nxt[:, PADC + c:PADC + c + n], ps[:, :n],
                                mybir.ActivationFunctionType.Square,
                                bias=neg_half[:])
                    if nxt_r:
                        # strip [x_late-3, xs): rounded copy of opB's output
                        nc.scalar.copy(
                            sqr[h][(s + 1) % 2][:, 1:1 + (xs - x_late + 3)],
                            nxt[:, PADC + x_late - 3:PADC + xs])

            if loop_k is not None:
                with tc.For_i(0, loop_k):
                    for s in range(steps):
                        emit_step(s, final=False)
            else:
                for s in range(steps):
                    emit_step(s, final=(s == steps - 1))

            fin = [sqq[h][steps % 2] for h in range(HALVES)]
            for h in range(HALVES):
                nc.vector.tensor_scalar(
                    fin[h][:, PADC:PADC + T], fin[h][:, PADC:PADC + T],
                    CLAMP, 1.0 - CLAMP,
                    mybir.AluOpType.max, mybir.AluOpType.min)
                nc.sync.dma_start(out=out_h[h], in_=fin[h][:, PADC:PADC + T])

    nc.compile()
    return nc


def get_nc(steps: int):
    if steps not in _compiled:
        _compiled[steps] = _build(steps)
    return _compiled[steps]


# --- host prep --------------------------------------------------------------

def _solve_za(C0, C1, C2, C3):
    """Per-channel real root of the zA-factorization cubic."""
    n = len(C0)
    mus = np.zeros(n); nus = np.zeros(n); bets = np.zeros(n)
    for c in range(n):
        a3, a2, a1, a0 = C3[c], C2[c], C1[c], C0[c]
        coeffs = [a3 ** 2, -2 * a2 * a3, a2 ** 2 + a1 * a3, a3 * a0 - a1 * a2]
        best = None
        for rt in np.roots(coeffs):
            if abs(rt.imag) > 1e-9 * max(1.0, abs(rt.real)):
                continue
            mu = rt.real
            b_ = a2 - a3 * mu
            if abs(b_) < 1e-9 or abs(mu) < 1e-7:
                continue
            nu = a0 / b_
            cond = abs(mu) + abs(nu) + abs(b_ / a3) + abs(nu / mu)
            if best is None or cond < best[0]:
                best = (cond, mu, nu, b_)
        assert best is not None, f"no usable root for channel {c}"
        _, mus[c], nus[c], bets[c] = best
    return mus, nus, bets


def _host_prep(drive, r, eps, beta, K_causal):
    """Per-core inputs: x (256,T), coef (256,8), wdiag (256,640)."""
    f32 = np.float32
    drive = np.asarray(drive, f32)
    r = np.asarray(r, np.float64)
    eps = np.asarray(eps, np.float64)
    beta = np.asarray(beta, np.float64)
    K = np.asarray(K_causal, np.float64)[:, 0, :]  # (C, 4)

    one_m_b = 1.0 - beta
    C0 = one_m_b * eps * r * K[:, 0]
    C1 = one_m_b * eps * r * K[:, 1]
    C2 = one_m_b * eps * r * K[:, 2]
    C3 = one_m_b * r * ((1.0 - eps) + eps * K[:, 3])
    dconst = 0.25 * (C0 + C1 + C2 + C3)
    mus, nus, bets = _solve_za(C0, C1, C2, C3)

    mu32 = mus.astype(f32)
    nom32 = (nus.astype(f32) / mus.astype(f32)).astype(f32)
    al32 = C3.astype(f32)
    boa32 = (bets.astype(f32) / C3.astype(f32)).astype(f32)
    # device-exact pad fixed point: D'pad = fl(P_pad) - 0.5
    q = f32(0.25)
    m1p = (mu32 * q).astype(f32)
    m2p = (nom32 * m1p).astype(f32)
    Zp = ((q + m1p).astype(f32) + m2p).astype(f32)
    mp = (al32 * Zp).astype(f32)
    wp = (boa32 * mp).astype(f32)
    Pp = (wp + mp).astype(f32)
    dpad = (Pp - f32(0.5)).astype(f32)

    in_maps = []
    idx = np.arange(128)
    for i in range(N_CORES):
        sl = slice(i * CPC, (i + 1) * CPC)
        xs = np.ascontiguousarray(
            drive[:, :, sl].transpose(0, 2, 1).reshape(ROWS, T), f32)
        cs = np.stack(
            [np.tile(mu32[sl], B), np.tile(nom32[sl], B),
             np.tile(al32[sl], B), np.tile(boa32[sl], B),
             np.tile(beta[sl].astype(f32), B),
             np.tile((dconst.astype(f32) - f32(0.5))[sl], B),
             np.tile(dpad[sl], B), np.tile(dconst.astype(f32)[sl], B),
             np.full(ROWS, CLAMP, f32), np.full(ROWS, 1.0 - CLAMP, f32)],
            axis=1).astype(f32)
        blocks = [-C3, -C2, -C1, -C0, np.ones(C)]
        wdg = np.zeros((ROWS, 640), f32)
        for k, arr in enumerate(blocks):
            rows = np.tile(arr.astype(f32)[sl], B)
            for h in range(HALVES):
                wdg[h * 128 + idx, k * 128 + idx] = rows[h * 128 + idx]
        in_maps.append({"x": xs, "coef": np.ascontiguousarray(cs),
                        "wdiag": wdg})
    return in_maps


def kernel(drive, r, eps, beta, K_causal, steps):
    steps = int(steps)
    nc = get_nc(steps)
    in_maps = _host_prep(drive, r, eps, beta, K_causal)
    res = run_bass_kernel_spmd(nc, in_maps, list(range(N_CORES)))
    parts = [
        res.results[i]["out"].reshape(B, CPC, T).transpose(0, 2, 1)
        for i in range(N_CORES)
    ]
    return np.ascontiguousarray(np.concatenate(parts, axis=2), np.float32)
